# revision 91
# baseline (speedup 1.0000x reference)
"""Longformer-style windowed self-attention for TRN2, 8-core SPMD.

Sharding: 24 (batch, head) pairs -> 3 heads per core (core c gets batch c//4,
heads (c%4)*3 .. +3). Each core computes QKV projections for its head slice,
windowed attention (block 256, window +-256), and writes its [4096, 192]
output channel slice. Host gathers slices into the full [2, 4096, 768] output.

Layout: all matmul operands bf16 (full-rate, 2x DVE packed mode vs f32).
Scores are computed transposed ([keys, queries]) into a compact gapless PSUM
layout of 5x256 columns per block; fully-masked half-chunks at the window
edges are skipped. Probabilities (exp on ACT) feed PV matmuls oriented
lhsT=probs [keys, q], rhs=[v|ones] so the output lands directly in [q, dh]
layout with the softmax denominator in an extra column - no PE transposes.
Band masks are bf16 multiplies on DVE, as are the PSUM->SBUF copies
(GPSIMD cannot access PSUM).

Scheduling (every engine queue is strictly in-order, so each consumer must
reuse only resources drained a full pipeline step earlier): per step, emit
scores/exp of block n heads A,B; then a mid section with 2-3 projection
matmul groups (earliest-deadline-first spread across all steps, hst DMAs
prefetched one step ahead) followed by PV+epilogue of block n-1; then head
C and all band masks at step end so DVE never blocks on the ACT exp chain.
Startup interleaves split weight/data DMAs; the final block interleaves
per-head PV with its masks and issues its output DMAs from the ACT queue.
"""

import sys

for _p in ("/opt/trn_rl_repo", "/opt/pypackages"):
    if _p not in sys.path:
        sys.path.append(_p)

import numpy as np
import ml_dtypes
from contextlib import ExitStack

import concourse.bass as bass
import concourse.bacc as bacc
import concourse.mybir as mybir
import concourse.tile as tile
from concourse.bass_utils import run_bass_kernel_spmd

F32 = mybir.dt.float32
BF16 = mybir.dt.bfloat16
EXP = mybir.ActivationFunctionType.Exp
NP_BF16 = ml_dtypes.bfloat16

B, S, D = 2, 4096, 768
H, DH = 12, 64
W = 256                 # one-sided window / query block size
NB = S // W             # 16 query blocks
NKC = S // 128          # 32 key chunks of 128
HPC = 3                 # heads per core
N_CORES = 8

# Column slot of window-chunk j in the compact per-block score layout.
# j1..j4 are full 256-wide; j0 keeps only its first q-half (queries 0:128),
# j5 only its second q-half. Slots are arranged gapless and bank-aligned:
# bank0 = [j1 | j2], bank1 = [j3 | j4], bank2 = [j0h0 | j5h1].
SLOT = {0: 1024, 1: 0, 2: 256, 3: 512, 4: 768, 5: 1152}
PSW = 1280  # score/prob tile width


def _chunks(n):
    """Present (m, j) chunk pairs for query block n."""
    out = []
    for j in range(6):
        m = 2 * n - 2 + j
        if 0 <= m < NKC:
            out.append((m, j))
    return out


def build_program(has_bias, has_kmask):
    nc = bacc.Bacc("TRN2", target_bir_lowering=False, debug=False,
                   num_devices=N_CORES)
    hsT_d = nc.declare_dram_parameter("hsT", [D, S], BF16, isOutput=False)
    wqk_d = nc.declare_dram_parameter("wqk", [D, 384], BF16, isOutput=False)
    wv_d = nc.declare_dram_parameter("wv", [D, 192], BF16, isOutput=False)
    msk_d = nc.declare_dram_parameter("masks", [128, 256], BF16, isOutput=False)
    if has_bias:
        bqk_d = nc.declare_dram_parameter("bqk", [1, 384], BF16, isOutput=False)
        bv_d = nc.declare_dram_parameter("bv", [1, 192], BF16, isOutput=False)
    if has_kmask:
        kpad_d = nc.declare_dram_parameter("kpad", [128, NKC], F32, isOutput=False)
        qpad_d = nc.declare_dram_parameter("qpad", [128, NKC], F32, isOutput=False)
    out_d = nc.declare_dram_parameter("out", [S, HPC * DH], F32, isOutput=True)

    with tile.TileContext(nc) as tc, ExitStack() as ctx:
        const_p = ctx.enter_context(tc.tile_pool(name="const", bufs=1))
        hst_p = ctx.enter_context(tc.tile_pool(name="hst", bufs=4))
        qkt_p = ctx.enter_context(tc.tile_pool(name="qkt", bufs=1))
        vall_p = ctx.enter_context(tc.tile_pool(name="vall", bufs=1))
        stg_p = ctx.enter_context(tc.tile_pool(name="stg", bufs=3))
        pt_p = ctx.enter_context(tc.tile_pool(name="pt", bufs=8))
        wk_p = ctx.enter_context(tc.tile_pool(name="wk", bufs=4))
        osb_p = ctx.enter_context(tc.tile_pool(name="osb", bufs=6))
        ps_p = ctx.enter_context(tc.tile_pool(name="ps", bufs=2, space="PSUM"))
        sm_p = ctx.enter_context(tc.tile_pool(name="sm", bufs=2, space="PSUM"))

        # ---- constants / weights ----
        # wqk/wv DMAs are emitted inside the first proj tile (interleaved
        # per-chunk with hst) so the first matmul starts ~1us in.
        wqk_sb = const_p.tile([128, 6, 384], BF16)
        wv_sb = const_p.tile([128, 6, 192], BF16)
        msk_sb = const_p.tile([128, 256], BF16)
        if has_bias:
            bqk_sb = const_p.tile([1, 384], BF16)
            nc.sync.dma_start(bqk_sb[:], bqk_d[:, :])
            bv_sb = const_p.tile([1, 192], BF16)
            nc.sync.dma_start(bv_sb[:], bv_d[:, :])
            ones_sb = const_p.tile([1, 512], BF16)
            nc.vector.memset(ones_sb[:], 1.0)
        if has_kmask:
            kpad_sb = const_p.tile([128, NKC], F32)
            nc.sync.dma_start(kpad_sb[:], kpad_d[:, :])
            qpad_sb = const_p.tile([128, NKC], F32)
            nc.sync.dma_start(qpad_sb[:], qpad_d[:, :])

        # qT/kT for head pair (A,B): A on partitions 0:64, B on 64:128
        qt_ab = qkt_p.tile([128, S], BF16)
        kt_ab = qkt_p.tile([128, S], BF16)
        # solo head C on partitions 0:64
        qt_c = qkt_p.tile([64, S], BF16)
        kt_c = qkt_p.tile([64, S], BF16)
        # v keyed by s on partitions: [128, key-chunk, head, (v | 1)]
        vall = vall_p.tile([128, NKC, 3, 65], BF16)
        ones_cols = vall[:].rearrange("p m h x -> p (m h) x")[:, :, 64:65]
        nc.vector.memset(ones_cols, 1.0)

        hst_tiles = {}

        def emit_hst_dma(t, startup=False):
            s0 = 512 * t
            hst = hst_p.tile([128, 6, 512], BF16, tag="hst", name="hst")
            hst_tiles[t] = hst
            src = hsT_d[:].rearrange("(c p) s -> p c s", p=128)[
                :, :, s0 : s0 + 512
            ]
            wsrc = wqk_d[:].rearrange("(c p) n -> p c n", p=128)
            if startup:
                # split transfers, weights interleaved with data, so the
                # first matmuls start after ~2 small transfers
                splits = ((0, 2), (2, 2), (4, 2)) if t == 0 else ((0, 3), (3, 3))
                for c0, cw in splits:
                    if t == 0:
                        nc.sync.dma_start(
                            wqk_sb[:, c0 : c0 + cw, :], wsrc[:, c0 : c0 + cw, :]
                        )
                    nc.sync.dma_start(
                        hst[:, c0 : c0 + cw, :], src[:, c0 : c0 + cw, :]
                    )
            else:
                nc.sync.dma_start(hst[:], src)

        def emit_qk_group(t, j, startup=False):
            s0 = 512 * t
            if j == 0 and t not in hst_tiles:
                emit_hst_dma(t, startup=startup)
            hst = hst_tiles[t]
            pp = sm_p.tile([128, 512], F32, space="PSUM", tag="sm")
            for c in range(6):
                nc.tensor.matmul(
                    pp[:],
                    (wqk_sb[:, c, 128 * j : 128 * j + 128]),
                    (hst[:, c, :]),
                    start=(c == 0),
                    stop=(c == 5 and not has_bias),
                )
            if has_bias:
                nc.tensor.matmul(
                    pp[:],
                    (bqk_sb[0:1, 128 * j : 128 * j + 128]),
                    (ones_sb[0:1, :]),
                    start=False,
                    stop=True,
                )
            if j == 0:
                nc.vector.tensor_copy(qt_ab[:, s0 : s0 + 512], pp[:])
            elif j == 1:
                nc.vector.tensor_copy(kt_ab[:, s0 : s0 + 512], pp[:])
            else:
                nc.vector.tensor_copy(qt_c[:, s0 : s0 + 512], pp[0:64, :])
                kstg = stg_p.tile([128, 512], BF16, tag="kstg")
                nc.vector.tensor_copy(kstg[64:128, :], pp[64:128, :])
                nc.sync.dma_start(kt_c[:, s0 : s0 + 512], kstg[64:128, :])

        def emit_v_part(t, mm0):
            s0 = 512 * t
            hst = hst_tiles.pop(t) if mm0 == 2 else hst_tiles[t]
            if t == 0 and mm0 == 0:
                nc.sync.dma_start(
                    wv_sb[:], wv_d[:].rearrange("(c p) n -> p c n", p=128)
                )
                nc.sync.dma_start(msk_sb[:], msk_d[:, :])
            # v projection: 2 s-subtiles of 128 in one PSUM tile, N=192
            pv = sm_p.tile([128, 512], F32, space="PSUM", tag="sm")
            for half, mm in enumerate((mm0, mm0 + 1)):
                for c in range(6):
                    nc.tensor.matmul(
                        pv[:, 256 * half : 256 * half + 192],
                        (hst[:, c, 128 * mm : 128 * mm + 128]),
                        (wv_sb[:, c, :]),
                        start=(c == 0),
                        stop=(c == 5 and not has_bias),
                    )
                if has_bias:
                    nc.tensor.matmul(
                        pv[:, 256 * half : 256 * half + 192],
                        (ones_sb[0:1, 0:128]),
                        (bv_sb[0:1, :]),
                        start=False,
                        stop=True,
                    )
            m = 4 * t + mm0
            dst = vall[:, m : m + 2, :, 0:64]
            src = pv[:].rearrange("p (m x) -> p m x", m=2)[
                :, :, 0:192
            ].rearrange("p m (h x) -> p m h x", h=3)
            nc.vector.tensor_copy(dst, src)

        def emit_proj_qk(t, startup=False):
            for j in range(3):
                emit_qk_group(t, j, startup=startup)

        def emit_proj_v(t):
            emit_v_part(t, 0)
            emit_v_part(t, 2)

        pts = {}  # block -> (chunk list, [pt_a, pt_b, pt_c])

        def emit_front(n, mid=None, tail=False):
            """Scores + exp for all 3 heads of block n; band masks at the end
            of the step so DVE's in-order queue can't block earlier work on
            the ACT exp chain. `mid` (PV of n-1, epilogue, proj part) runs
            between heads B and C to absorb the exp-A PSUM-drain latency."""
            q0 = 256 * n
            ch = _chunks(n)
            have = {j for _, j in ch}
            lo = min(SLOT[j] for _, j in ch)
            hi = max(SLOT[j] + (128 if j in (0, 5) else 256) for _, j in ch)

            def emit_scores(ps, kt, qt, tp):
                for m, j in ch:
                    if j == 0:
                        qs, qw = q0, 128
                    elif j == 5:
                        qs, qw = q0 + 128, 128
                    else:
                        qs, qw = q0, 256
                    nc.tensor.matmul(
                        ps[:, SLOT[j] : SLOT[j] + qw],
                        (kt[:, 128 * m : 128 * m + 128]),
                        (qt[:, qs : qs + qw]),
                        start=True,
                        stop=True,
                        tile_position=tp,
                    )

            def emit_exp(pt, ps):
                # exp over the live extent; edge blocks exclude dead slots
                if n == 0:
                    nc.scalar.activation(pt[:, 256:1024], ps[:, 256:1024], EXP)
                    nc.scalar.activation(pt[:, 1152:1280], ps[:, 1152:1280], EXP)
                elif n == NB - 1:
                    nc.scalar.activation(pt[:, 0:768], ps[:, 0:768], EXP)
                    nc.scalar.activation(pt[:, 1024:1152], ps[:, 1024:1152], EXP)
                else:
                    nc.scalar.activation(pt[:, lo:hi], ps[:, lo:hi], EXP)

            def emit_masks(pt):
                # band masks: L = keep q<=r, U = keep q>=r
                if 0 in have and 5 in have:
                    nc.vector.tensor_mul(
                        pt[:, 1024:1280], pt[:, 1024:1280], msk_sb[:, 0:256]
                    )
                elif 0 in have:
                    nc.vector.tensor_mul(
                        pt[:, 1024:1152], pt[:, 1024:1152], msk_sb[:, 0:128]
                    )
                elif 5 in have:
                    nc.vector.tensor_mul(
                        pt[:, 1152:1280], pt[:, 1152:1280], msk_sb[:, 128:256]
                    )
                if 1 in have and 4 in have:
                    # j1h1 (L at 128) + j4h0 (U at 768) as one strided op
                    v = pt[:, 128:1024].rearrange("p (a w) -> p a w", a=7)[:, ::5, :]
                    mk = msk_sb[:].rearrange("p (a w) -> p a w", a=2)
                    nc.vector.tensor_mul(v, v, mk)
                elif 1 in have:
                    nc.vector.tensor_mul(
                        pt[:, 128:256], pt[:, 128:256], msk_sb[:, 0:128]
                    )
                elif 4 in have:
                    nc.vector.tensor_mul(
                        pt[:, 768:896], pt[:, 768:896], msk_sb[:, 128:256]
                    )
                if has_kmask:
                    for m, j in ch:
                        w = 128 if j in (0, 5) else 256
                        nc.vector.tensor_scalar_mul(
                            pt[:, SLOT[j] : SLOT[j] + w],
                            pt[:, SLOT[j] : SLOT[j] + w],
                            kpad_sb[:, m : m + 1],
                        )

            ps_a = ps_p.tile([128, PSW], F32, space="PSUM", tag="ps")
            emit_scores(ps_a, kt_ab[0:64, :], qt_ab[0:64, :], (0, 0))
            pt_a = pt_p.tile([128, PSW], BF16, tag="pt")
            emit_exp(pt_a, ps_a)
            ps_b = ps_p.tile([128, PSW], F32, space="PSUM", tag="ps")
            emit_scores(ps_b, kt_ab[64:128, :], qt_ab[64:128, :], (64, 0))
            pt_b = pt_p.tile([128, PSW], BF16, tag="pt")
            emit_exp(pt_b, ps_b)
            if mid is not None:
                mid()
            ps_c = ps_p.tile([128, PSW], F32, space="PSUM", tag="ps")
            emit_scores(ps_c, kt_c[:, :], qt_c[:, :], (0, 0))
            pt_c = pt_p.tile([128, PSW], BF16, tag="pt")
            emit_exp(pt_c, ps_c)
            hpts = [(pt_a, 0), (pt_b, 1), (pt_c, 2)]
            if not tail:
                for pt, _ in hpts:
                    emit_masks(pt)
                pts[n] = (ch, hpts)
            else:
                # final block: interleave each head's PV right after its mask;
                # finish q-half 0 entirely first so its epilogue + output DMA
                # overlap the q-half-1 PV matmuls
                outs = [
                    sm_p.tile([128, 512], F32, space="PSUM", tag="sm", name="ot")
                    for _ in range(2)
                ]
                for pt, h in hpts:
                    emit_masks(pt)
                    emit_pv_group(n, ch, outs[0], pt, h, 0)
                emit_epilogue(n, outs[0], 0, dma_engine=nc.scalar)
                for pt, h in hpts:
                    emit_pv_group(n, ch, outs[1], pt, h, 1)
                emit_epilogue(n, outs[1], 1, dma_engine=nc.scalar)

        def emit_pv_group(n, ch, outp, pt, h, g):
            # edge chunks last: their pt slices come from the later exp call,
            # so the accumulation group can start on the full chunks first
            pvch = sorted(
                ((m, j) for m, j in ch
                 if (j != 0 or g == 0) and (j != 5 or g == 1)),
                key=lambda mj: mj[1] in (0, 5),
            )
            base = 65 * h
            for oi, (m, j) in enumerate(pvch):
                c0 = SLOT[j] if j in (0, 5) else SLOT[j] + 128 * g
                nc.tensor.matmul(
                    outp[:, base : base + 65],
                    (pt[:, c0 : c0 + 128]),
                    (vall[:, m, h, 0:65]),
                    start=(oi == 0),
                    stop=(oi == len(pvch) - 1),
                )

        def emit_epilogue(n, outp, g, dma_engine=None):
            q0 = 256 * n
            rec = wk_p.tile([128, 3], F32, name="rec", tag="rec")
            dcol = outp[:, 0:195].rearrange("p (s x) -> p s x", x=65)[:, :, 64:65]
            nc.vector.reciprocal(rec[:].rearrange("p (s x) -> p s x", x=1), dcol)
            osb = osb_p.tile([128, 192], F32, tag="osb")
            for h in range(3):
                nc.vector.tensor_scalar_mul(
                    osb[:, 64 * h : 64 * h + 64],
                    outp[:, 65 * h : 65 * h + 64],
                    rec[:, h : h + 1],
                )
            if has_kmask:
                nc.vector.tensor_scalar_mul(
                    osb[:], osb[:], qpad_sb[:, 2 * n + g : 2 * n + g + 1]
                )
            (dma_engine or nc.sync).dma_start(
                out_d[q0 + 128 * g : q0 + 128 * g + 128, 0:192], osb[:]
            )

        def emit_back(n):
            """PV + normalize + output DMA for block n."""
            ch, hpts = pts.pop(n)
            for g in (1, 0):
                # own PSUM tile per q-half so this half's normalize (DVE read)
                # overlaps the other half's PV matmuls (different bank)
                outp = sm_p.tile([128, 512], F32, space="PSUM", tag="sm")
                for pt, h in hpts:
                    emit_pv_group(n, ch, outp, pt, h, g)
                emit_epilogue(n, outp, g)

        # Software pipeline: front(n) = scores/exp/mask, then back(n-1) = PV +
        # epilogue one block behind (pt fully drained by then), proj tiles
        # interleaved every other block.
        emit_proj_qk(0, startup=True)
        emit_v_part(0, 0)
        emit_proj_qk(1, startup=True)
        emit_front(0)

        # step -> proj groups, spread ~2 per step (earliest-deadline-first) so
        # PE filler is even and covers the late steps; qk(t) is due before
        # front(2t-1), v(t) before back(2t-1) in mid(2t)
        PROJ_AT = {
            1: [("v", 0, 2), ("v", 1, 0), ("qk", 2, 0)],
            2: [("hst", 3, 0), ("qk", 2, 1), ("qk", 2, 2), ("v", 1, 2)],
            3: [("qk", 3, 0), ("qk", 3, 1)],
            4: [("hst", 4, 0), ("qk", 3, 2), ("v", 2, 0)],
            5: [("v", 2, 2), ("qk", 4, 0)],
            6: [("hst", 5, 0), ("qk", 4, 1), ("qk", 4, 2), ("v", 3, 0)],
            7: [("v", 3, 2), ("qk", 5, 0)],
            8: [("hst", 6, 0), ("qk", 5, 1), ("qk", 5, 2), ("v", 4, 0)],
            9: [("v", 4, 2), ("qk", 6, 0)],
            10: [("hst", 7, 0), ("qk", 6, 1), ("qk", 6, 2), ("v", 5, 0)],
            11: [("v", 5, 2), ("qk", 7, 0)],
            12: [("qk", 7, 1), ("qk", 7, 2), ("v", 6, 0)],
            13: [("v", 6, 2)],
            14: [("v", 7, 0)],
            15: [("v", 7, 2)],
        }

        def mk_mid(n):
            def mid():
                # proj first: its PSUM tiles then reuse buffers drained by the
                # PREVIOUS step's epilogue, and this step's outp tiles reuse
                # buffers drained by the proj copies just emitted - every
                # consumer sees a one-step-old, already-drained buffer
                for kind, t, idx in PROJ_AT.get(n, ()):
                    if kind == "qk":
                        emit_qk_group(t, idx)
                    elif kind == "hst":
                        emit_hst_dma(t)
                    else:
                        emit_v_part(t, idx)
                emit_back(n - 1)
            return mid

        for n in range(1, NB - 1):
            emit_front(n, mid=mk_mid(n))
        emit_front(NB - 1, mid=mk_mid(NB - 1), tail=True)

    nc.compile()
    return nc


_prog_cache = {}


def _get_program(has_bias, has_kmask):
    key = (has_bias, has_kmask)
    if key not in _prog_cache:
        _prog_cache[key] = build_program(has_bias, has_kmask)
    return _prog_cache[key]


def _band_masks():
    """[L | U] keep-masks: L = q<=r, U = q>=r on [128, 128]."""
    r = np.arange(128)[:, None]
    q = np.arange(128)[None, :]
    lo = (q <= r).astype(np.float32)
    up = (q >= r).astype(np.float32)
    return np.concatenate([lo, up], axis=1)


def kernel(hidden_states, attention_mask, Wq, bq, Wk, bk, Wv, bv, _res=[None]):
    hidden_states = np.asarray(hidden_states, np.float32)
    attention_mask = np.asarray(attention_mask, np.float32)
    Wq, Wk, Wv = (np.asarray(w, np.float32) for w in (Wq, Wk, Wv))
    bq, bk, bv = (np.asarray(b_, np.float32) for b_ in (bq, bk, bv))

    scale = 1.0 / np.sqrt(DH)
    has_bias = bool(np.any(bq) or np.any(bk) or np.any(bv))
    has_kmask = bool(np.any(attention_mask < 0))

    hsT = [
        np.ascontiguousarray(hidden_states[b].T).astype(NP_BF16) for b in range(B)
    ]
    masks = _band_masks().astype(NP_BF16)
    masked = attention_mask < 0  # [B, S]

    in_maps = []
    for core in range(N_CORES):
        b, h0 = core // 4, (core % 4) * HPC
        sl = slice(h0 * DH, (h0 + HPC) * DH)
        wq = Wq[:, sl] * scale
        wk = Wk[:, sl]
        wqk = np.concatenate(
            [wq[:, 0:128], wk[:, 0:128], wq[:, 128:192], wk[:, 128:192]], axis=1
        )
        m = {
            "hsT": hsT[b],
            "wqk": np.ascontiguousarray(wqk).astype(NP_BF16),
            "wv": np.ascontiguousarray(Wv[:, sl]).astype(NP_BF16),
            "masks": masks,
        }
        if has_bias:
            bq_s = bq[sl] * scale
            bk_s = bk[sl]
            m["bqk"] = np.concatenate(
                [bq_s[0:128], bk_s[0:128], bq_s[128:192], bk_s[128:192]]
            ).reshape(1, 384).astype(NP_BF16)
            m["bv"] = bv[sl].reshape(1, 192).astype(NP_BF16)
        if has_kmask:
            keep = (~masked[b]).astype(np.float32).reshape(NKC, 128).T
            m["kpad"] = np.ascontiguousarray(keep)
            m["qpad"] = np.ascontiguousarray(keep)
        in_maps.append(m)

    nc = _get_program(has_bias, has_kmask)
    res = run_bass_kernel_spmd(nc, in_maps, list(range(N_CORES)))
    _res[0] = res

    out = np.empty((B, S, D), np.float32)
    for core in range(N_CORES):
        b, h0 = core // 4, (core % 4) * HPC
        out[b, :, h0 * DH : (h0 + HPC) * DH] = res.results[core]["out"]
    return out
